# revision 18
# baseline (speedup 1.0000x reference)
"""Trainium2 Bass kernel for the CPC loss (nn_CPC_292057776614).

Strategy (data-parallel over the prediction axis, 8 cores):
  - The 8960 predictions split into 5 step segments (lengths 2688, 2240,
    1792, 1344, 896); each core takes a contiguous 1/8 of every segment
    -> 1120 predictions/core, padded to 1152 = 9 tiles of 128.
  - Per core, on device:
      1. indirect-DMA gather of the 1152 context rows (f32), cast to
         bf16, PE-transpose into ctxT [D, 1152].
      2. stage A: predT = (Wk_w[s]^T)-contraction of ctxT accumulated
         over 10 k-tiles in PSUM, + bias, cast to bf16.
      3. stage B: all-pairs scores = predT.T @ encT -> [1152, 3136] f32
         in PSUM chunks.
      4. masked softmax directly against the scores chunks using three
         host-built index-plan masks (additive -1e30 mask N, candidate
         multiplicity M, positive one-hot P0):
            masked = scores + N
            truemax = max_e masked          (exact candidate max)
            dots0  = sum_e P0 * masked      (exact positive logit)
            sumexp = sum_e M * exp(masked - truemax)
            loss_p = ln(sumexp) + truemax - dots0
            corr_p = dots0 >= truemax
         This avoids any per-element gather (TRN2 indirect DMA only
         supports one index per partition).
      5. masked partial sums reduced across partitions with a
         ones-vector matmul -> [1, 2] per core.
  - Host sums the 8 per-core [loss_sum, correct_sum] pairs and divides
    by 8960.
"""

import numpy as np
import ml_dtypes

import concourse.bass as bass
import concourse.mybir as mybir
import concourse.tile as tile
from concourse import bacc
from concourse.bass import IndirectOffsetOnAxis
from concourse.bass_utils import run_bass_kernel_spmd
from concourse.masks import make_identity

BF16 = mybir.dt.bfloat16
F32 = mybir.dt.float32
I32 = mybir.dt.int32

# Problem constants (hardcoded; kernel.py must be self-contained).
B, G, D, S, NEG = 64, 7, 1280, 5, 16
CELLS = G * G            # 49
R = B * CELLS            # 3136 rows in ctx/enc
K17 = NEG + 1            # 17 candidates per prediction
STEP_LENS = [B * (G - 1 - s) * G for s in range(S)]     # [2688,2240,1792,1344,896]
P_TOTAL = sum(STEP_LENS)                                # 8960
N_CORES = 8
L = [sl // N_CORES for sl in STEP_LENS]                 # [336,280,224,168,112]
PC = sum(L)                                             # 1120 per core
NT = 9                                                  # p-tiles of 128
PP = NT * 128                                           # 1152 padded
PO = [sum(L[:s]) for s in range(S)]                     # per-core step offsets
KD = D // 128                                           # 10 k-tiles
ECH = 448                                               # e-chunk width (448*7=3136)
NE = R // ECH                                           # 7 chunks
NEGINF = -1.0e30

_CACHE = {}

DEBUG = bool(int(__import__("os").environ.get("BASS_CPC_DEBUG", "0")))


def _build():
    """Build (and cache) the per-core Bass program. All 8 cores run the
    identical program on different data."""
    if "nc" in _CACHE:
        return _CACHE["nc"]

    nc = bacc.Bacc("TRN2", target_bir_lowering=False, debug=False)

    ctx_d = nc.dram_tensor("ctx", [R, D], BF16, kind="ExternalInput")
    encT_d = nc.dram_tensor("encT", [D, R], BF16, kind="ExternalInput")
    WT_d = nc.dram_tensor("WT", [S, D, D], BF16, kind="ExternalInput")
    bias_d = nc.dram_tensor("biasT", [128, S * KD], F32, kind="ExternalInput")
    gidx_d = nc.dram_tensor("gidx", [128, NT], I32, kind="ExternalInput")
    vmask_d = nc.dram_tensor("vmask", [128, NT], F32, kind="ExternalInput")
    nmask_d = nc.dram_tensor("nmask", [PP, R], BF16, kind="ExternalInput")
    mmask_d = nc.dram_tensor("mmask", [PP, R], BF16, kind="ExternalInput")
    pmask_d = nc.dram_tensor("pmask", [PP, R], BF16, kind="ExternalInput")
    out_d = nc.dram_tensor("out", [1, 2], F32, kind="ExternalOutput")
    if DEBUG:
        predT_dbg = nc.dram_tensor(
            "predT_dbg", [128, KD, PP], BF16, kind="ExternalOutput"
        )
        cols_dbg = nc.dram_tensor("cols_dbg", [128, 4 * NT], F32, kind="ExternalOutput")
        res_dbg = nc.dram_tensor("res_dbg", [128, 2 * NT], F32, kind="ExternalOutput")

    with tile.TileContext(nc) as tc:
        with (
            tc.tile_pool(name="const", bufs=1) as const,
            tc.tile_pool(name="spool", bufs=4) as spool,
            tc.tile_pool(name="ps", bufs=4, space="PSUM") as ps_pool,
            tc.tile_pool(name="pt", bufs=2, space="PSUM") as pt_pool,
            tc.tile_pool(name="psf", bufs=1, space="PSUM") as psf_pool,
        ):
            # ---- constants / persistent tiles ----
            encT_sb = const.tile([128, KD, R], BF16)
            nc.sync.dma_start(
                out=encT_sb[:],
                in_=encT_d.ap().rearrange("(k p) e -> p k e", p=128),
            )
            bias_sb = const.tile([128, S * KD], F32)
            nc.sync.dma_start(out=bias_sb[:], in_=bias_d.ap())
            gidx_sb = const.tile([128, NT], I32)
            nc.sync.dma_start(out=gidx_sb[:], in_=gidx_d.ap())
            vmask_sb = const.tile([128, NT], F32)
            nc.sync.dma_start(out=vmask_sb[:], in_=vmask_d.ap())

            ident = const.tile([128, 128], BF16)
            make_identity(nc, ident[:])
            ones = const.tile([128, 1], F32)
            nc.vector.memset(ones[:], 1.0)

            ctxT_sb = const.tile([128, KD, PP], BF16)
            predT_sb = const.tile([128, KD, PP], BF16)
            # zero the padded prediction columns so stage B stays finite
            nc.vector.memset(predT_sb[:, :, PC:PP], 0.0)

            nmax_sb = const.tile([128, NT], F32)   # negated candidate max
            dots0_sb = const.tile([128, NT], F32)  # positive logit
            sume_sb = const.tile([128, NT], F32)   # sum of M*exp(masked-max)
            lnS_sb = const.tile([128, NT], F32)
            res_sb = const.tile([128, 2 * NT], F32)

            # ---- phase 1: gather ctx rows, cast, transpose ----
            with tc.tile_pool(name="gpool", bufs=3) as gpool:
                for t in range(NT):
                    g = gpool.tile([128, D], BF16)
                    nc.gpsimd.indirect_dma_start(
                        out=g[:],
                        out_offset=None,
                        in_=ctx_d.ap(),
                        in_offset=IndirectOffsetOnAxis(
                            ap=gidx_sb[:, t : t + 1], axis=0
                        ),
                    )
                    for k in range(KD):
                        pt = pt_pool.tile([128, 128], BF16, tag="pt")
                        nc.tensor.transpose(
                            pt[:], g[:, k * 128 : (k + 1) * 128], ident[:]
                        )
                        nc.vector.tensor_copy(
                            ctxT_sb[:, k, t * 128 : (t + 1) * 128], pt[:]
                        )

            # ---- phase 2 (stage A): predT = W^T-contract(ctxT) + bias ----
            with tc.tile_pool(name="wpool", bufs=2) as wpool:
                for s in range(S):
                    w_sb = wpool.tile([128, KD, D], BF16)
                    nc.sync.dma_start(
                        out=w_sb[:],
                        in_=WT_d.ap()[s].rearrange("(k p) j -> p k j", p=128),
                    )
                    lo, ln = PO[s], L[s]
                    for m in range(KD):
                        pa = ps_pool.tile([128, ECH], F32, tag="ps")
                        for k in range(KD):
                            nc.tensor.matmul(
                                pa[:, :ln],
                                lhsT=w_sb[:, k, m * 128 : (m + 1) * 128],
                                rhs=ctxT_sb[:, k, lo : lo + ln],
                                start=(k == 0),
                                stop=(k == KD - 1),
                            )
                        nc.scalar.activation(
                            predT_sb[:, m, lo : lo + ln],
                            pa[:, :ln],
                            mybir.ActivationFunctionType.Identity,
                            bias=bias_sb[:, s * KD + m : s * KD + m + 1],
                            scale=1.0,
                        )

            # ---- phase 3 (stage B + masked softmax stats per p-tile) ----
            with (
                tc.tile_pool(name="mpool", bufs=1) as mpool,
                tc.tile_pool(name="mk", bufs=2) as mkpool,
                tc.tile_pool(name="trash", bufs=1) as trashpool,
            ):
                for mp in range(NT):
                    rows = slice(mp * 128, (mp + 1) * 128)
                    Nt = mpool.tile([128, R], BF16, tag="N")
                    nc.sync.dma_start(out=Nt[:], in_=nmask_d.ap()[rows, :])
                    Mt = mpool.tile([128, R], BF16, tag="M")
                    nc.sync.dma_start(out=Mt[:], in_=mmask_d.ap()[rows, :])
                    Pt = mpool.tile([128, R], BF16, tag="P")
                    nc.sync.dma_start(out=Pt[:], in_=pmask_d.ap()[rows, :])

                    masked = mkpool.tile([128, R], F32)
                    for n in range(NE):
                        cols = slice(n * ECH, (n + 1) * ECH)
                        pb = ps_pool.tile([128, ECH], F32, tag="ps")
                        for k in range(KD):
                            nc.tensor.matmul(
                                pb[:],
                                lhsT=predT_sb[:, k, rows],
                                rhs=encT_sb[:, k, cols],
                                start=(k == 0),
                                stop=(k == KD - 1),
                            )
                        # evacuate + apply the -inf candidate mask
                        nc.vector.tensor_add(masked[:, cols], pb[:], Nt[:, cols])

                    # exact positive logit: sum_e P0 * masked
                    scr = trashpool.tile([128, R], F32, tag="scr")
                    nc.vector.tensor_mul(scr[:], masked[:], Pt[:])
                    nc.vector.reduce_sum(
                        dots0_sb[:, mp : mp + 1], scr[:], axis=mybir.AxisListType.X
                    )
                    # negated candidate max
                    nc.vector.reduce_max(
                        nmax_sb[:, mp : mp + 1],
                        masked[:],
                        axis=mybir.AxisListType.X,
                        negate=True,
                    )
                    # exp(masked - max) (scalar engine)
                    Et = trashpool.tile([128, R], BF16, tag="E")
                    nc.scalar.activation(
                        Et[:],
                        masked[:],
                        mybir.ActivationFunctionType.Exp,
                        bias=nmax_sb[:, mp : mp + 1],
                        scale=1.0,
                    )
                    # sumexp = sum_e M * exp(...)
                    nc.vector.tensor_mul(scr[:], Et[:], Mt[:])
                    nc.vector.reduce_sum(
                        sume_sb[:, mp : mp + 1], scr[:], axis=mybir.AxisListType.X
                    )

            # ---- phase 4: per-prediction loss/correct, masked, reduced ----
            nc.scalar.activation(
                lnS_sb[:], sume_sb[:], mybir.ActivationFunctionType.Ln
            )
            t1 = spool.tile([128, NT], F32)
            nc.vector.tensor_sub(t1[:], lnS_sb[:], dots0_sb[:])
            lossp = spool.tile([128, NT], F32)
            nc.vector.tensor_sub(lossp[:], t1[:], nmax_sb[:])  # + truemax
            tmax = spool.tile([128, NT], F32)
            nc.vector.tensor_scalar_mul(tmax[:], nmax_sb[:], -1.0)
            corrp = spool.tile([128, NT], F32)
            nc.vector.tensor_tensor(
                out=corrp[:], in0=dots0_sb[:], in1=tmax[:], op=mybir.AluOpType.is_ge
            )
            nc.vector.tensor_mul(res_sb[:, 0:NT], lossp[:], vmask_sb[:])
            nc.vector.tensor_mul(res_sb[:, NT : 2 * NT], corrp[:], vmask_sb[:])

            if DEBUG:
                nc.sync.dma_start(out=predT_dbg.ap(), in_=predT_sb[:])
                nc.sync.dma_start(out=cols_dbg.ap()[:, 0:NT], in_=nmax_sb[:])
                nc.sync.dma_start(
                    out=cols_dbg.ap()[:, NT : 2 * NT], in_=dots0_sb[:]
                )
                nc.sync.dma_start(
                    out=cols_dbg.ap()[:, 2 * NT : 3 * NT], in_=sume_sb[:]
                )
                nc.sync.dma_start(
                    out=cols_dbg.ap()[:, 3 * NT : 4 * NT], in_=lnS_sb[:]
                )
                nc.sync.dma_start(out=res_dbg.ap(), in_=res_sb[:])

            # ---- final reduction ----
            fin = const.tile([128, 2], F32)
            nc.vector.reduce_sum(
                fin[:, 0:1], res_sb[:, 0:NT], axis=mybir.AxisListType.X
            )
            nc.vector.reduce_sum(
                fin[:, 1:2], res_sb[:, NT : 2 * NT], axis=mybir.AxisListType.X
            )
            pf = psf_pool.tile([1, 2], F32)
            nc.tensor.matmul(pf[:], lhsT=ones[:], rhs=fin[:], start=True, stop=True)
            out_sb = const.tile([1, 2], F32)
            nc.vector.tensor_copy(out_sb[:], pf[:])
            nc.sync.dma_start(out=out_d.ap(), in_=out_sb[:])

    nc.compile()
    _CACHE["nc"] = nc
    return nc


def _prep_in_maps(contexts, encodings, Wk_w, Wk_b, ctx_idx, cand_idx):
    ctx_flat = np.ascontiguousarray(
        np.asarray(contexts, dtype=np.float32).reshape(R, D)
    ).astype(ml_dtypes.bfloat16)
    encT = np.ascontiguousarray(
        np.asarray(encodings, dtype=np.float32).reshape(R, D).T
    ).astype(ml_dtypes.bfloat16)
    WT = np.ascontiguousarray(
        np.asarray(Wk_w, dtype=np.float32).transpose(0, 2, 1)
    ).astype(ml_dtypes.bfloat16)
    biasT = np.ascontiguousarray(
        np.asarray(Wk_b, dtype=np.float32).reshape(S, KD, 128).transpose(2, 0, 1)
        .reshape(128, S * KD)
    )
    ctx_idx = np.asarray(ctx_idx, dtype=np.int32)
    cand_idx = np.asarray(cand_idx, dtype=np.int32)

    offs = np.concatenate([[0], np.cumsum(STEP_LENS)]).astype(np.int64)

    in_maps = []
    for c in range(N_CORES):
        ci_parts, ki_parts = [], []
        for s in range(S):
            a = int(offs[s]) + c * L[s]
            ci_parts.append(ctx_idx[a : a + L[s]])
            ki_parts.append(cand_idx[a : a + L[s]])
        ci = np.concatenate(ci_parts)                    # [1120]
        ki = np.concatenate(ki_parts, axis=0).astype(np.int64)  # [1120, 17]
        ci_pad = np.zeros(PP, np.int32)
        ci_pad[:PC] = ci
        gidx = np.ascontiguousarray(ci_pad.reshape(NT, 128).T)            # [128, 9]
        vmask = np.ascontiguousarray(
            (np.arange(PP) < PC).astype(np.float32).reshape(NT, 128).T
        )
        prow = np.arange(PC)
        nm = np.full((PP, R), NEGINF, np.float32)
        nm[prow[:, None], ki] = 0.0
        nm[PC:, 0] = 0.0
        mm = np.zeros((PP, R), np.float32)
        np.add.at(mm, (np.repeat(prow, K17), ki.ravel()), 1.0)
        mm[PC:, 0] = 1.0
        pm = np.zeros((PP, R), np.float32)
        pm[prow, ki[:, 0]] = 1.0
        pm[PC:, 0] = 1.0
        in_maps.append(
            {
                "ctx": ctx_flat,
                "encT": encT,
                "WT": WT,
                "biasT": biasT,
                "gidx": gidx,
                "vmask": vmask,
                "nmask": nm.astype(ml_dtypes.bfloat16),
                "mmask": mm.astype(ml_dtypes.bfloat16),
                "pmask": pm.astype(ml_dtypes.bfloat16),
            }
        )
    return in_maps


def _install_ntff_hook():
    """Provide antenv.axon_hooks if the image lacks it, so trace=True can
    capture NTFF profiles through the injected libaxon_pjrt.so (mirrors
    trn_boot._ntff_profile_via_ctypes)."""
    import sys
    import types
    import ctypes
    import contextlib
    import os

    try:
        from antenv.axon_hooks import get_axon_ntff_profile_hook  # noqa: F401

        return
    except ImportError:
        pass
    so_path = "/opt/axon/libaxon_pjrt.so"
    if not os.path.exists(so_path):
        return
    lib = ctypes.CDLL(so_path)
    if not hasattr(lib, "axon_start_nrt_profile"):
        return
    lib.axon_start_nrt_profile.argtypes = [
        ctypes.POINTER(ctypes.c_int64),
        ctypes.c_size_t,
    ]
    lib.axon_start_nrt_profile.restype = ctypes.c_int64
    lib.axon_stop_nrt_profile.argtypes = [ctypes.c_char_p]
    lib.axon_stop_nrt_profile.restype = ctypes.c_int64

    @contextlib.contextmanager
    def _hook(output_dir, device_ids):
        import jax

        jax.devices()
        if device_ids:
            ids = (ctypes.c_int64 * len(device_ids))(*device_ids)
            rc = lib.axon_start_nrt_profile(ids, len(device_ids))
        else:
            rc = lib.axon_start_nrt_profile(None, 0)
        if rc != 0:
            raise RuntimeError(f"axon_start_nrt_profile rc={rc}")
        try:
            yield
        finally:
            n = lib.axon_stop_nrt_profile(str(output_dir).encode())
            print(f"ntff profile: {n} file(s) written to {output_dir}")

    mod = types.ModuleType("antenv.axon_hooks")
    mod.get_axon_ntff_profile_hook = lambda: _hook
    mod.set_axon_ntff_profile_hook = lambda h: None
    sys.modules["antenv.axon_hooks"] = mod


def run(inputs, trace=False, **kwargs):
    """Run the SPMD kernel; returns (loss, correct, BassKernelResults)."""
    if trace:
        _install_ntff_hook()
    nc = _build()
    in_maps = _prep_in_maps(**inputs)
    res = run_bass_kernel_spmd(
        nc, in_maps, core_ids=list(range(N_CORES)), trace=trace, **kwargs
    )
    sums = np.stack([r["out"].reshape(2) for r in res.results])  # [8, 2]
    tot = sums.sum(axis=0, dtype=np.float64)
    loss = np.float32(tot[0] / P_TOTAL)
    correct = np.float32(tot[1] / P_TOTAL)
    return loss, correct, res


def kernel(**inputs):
    loss, correct, _ = run(inputs, trace=False)
    return loss, correct


# revision 20
# speedup vs baseline: 1.1179x; 1.1179x over previous
"""Trainium2 Bass kernel for the CPC loss (nn_CPC_292057776614).

Strategy (data-parallel over the prediction axis, 8 cores):
  - The 8960 predictions split into 5 step segments (lengths 2688, 2240,
    1792, 1344, 896); each core takes a contiguous 1/8 of every segment
    -> 1120 predictions/core, padded to 1152 = 9 tiles of 128.
  - Per core, on device:
      1. indirect-DMA gather of the 1152 context rows (f32), cast to
         bf16, PE-transpose into ctxT [D, 1152].
      2. stage A: predT = (Wk_w[s]^T)-contraction of ctxT accumulated
         over 10 k-tiles in PSUM, + bias, cast to bf16.
      3. stage B: all-pairs scores = predT.T @ encT -> [1152, 3136] f32
         in PSUM chunks.
      4. masked softmax directly against the scores chunks using three
         host-built index-plan masks (additive -1e30 mask N, candidate
         multiplicity M, positive one-hot P0):
            masked = scores + N
            truemax = max_e masked          (exact candidate max)
            dots0  = sum_e P0 * masked      (exact positive logit)
            sumexp = sum_e M * exp(masked - truemax)
            loss_p = ln(sumexp) + truemax - dots0
            corr_p = dots0 >= truemax
         This avoids any per-element gather (TRN2 indirect DMA only
         supports one index per partition).
      5. masked partial sums reduced across partitions with a
         ones-vector matmul -> [1, 2] per core.
  - Host sums the 8 per-core [loss_sum, correct_sum] pairs and divides
    by 8960.
"""

import numpy as np
import ml_dtypes

import concourse.bass as bass
import concourse.mybir as mybir
import concourse.tile as tile
from concourse import bacc
from concourse.bass import IndirectOffsetOnAxis
from concourse.bass_utils import run_bass_kernel_spmd
from concourse.masks import make_identity

BF16 = mybir.dt.bfloat16
F32 = mybir.dt.float32
I32 = mybir.dt.int32

# Problem constants (hardcoded; kernel.py must be self-contained).
B, G, D, S, NEG = 64, 7, 1280, 5, 16
CELLS = G * G            # 49
R = B * CELLS            # 3136 rows in ctx/enc
K17 = NEG + 1            # 17 candidates per prediction
STEP_LENS = [B * (G - 1 - s) * G for s in range(S)]     # [2688,2240,1792,1344,896]
P_TOTAL = sum(STEP_LENS)                                # 8960
N_CORES = 8
L = [sl // N_CORES for sl in STEP_LENS]                 # [336,280,224,168,112]
PC = sum(L)                                             # 1120 per core
NT = 9                                                  # p-tiles of 128
PP = NT * 128                                           # 1152 padded
PO = [sum(L[:s]) for s in range(S)]                     # per-core step offsets
KD = D // 128                                           # 10 k-tiles
ECH = 448                                               # e-chunk width (448*7=3136)
NE = R // ECH                                           # 7 chunks
NEGINF = -1.0e30

_CACHE = {}

DEBUG = bool(int(__import__("os").environ.get("BASS_CPC_DEBUG", "0")))


def _build():
    """Build (and cache) the per-core Bass program. All 8 cores run the
    identical program on different data."""
    if "nc" in _CACHE:
        return _CACHE["nc"]

    nc = bacc.Bacc("TRN2", target_bir_lowering=False, debug=False)

    ctx_d = nc.dram_tensor("ctx", [R, D], BF16, kind="ExternalInput")
    encT_d = nc.dram_tensor("encT", [D, R], BF16, kind="ExternalInput")
    WT_d = nc.dram_tensor("WT", [S, D, D], BF16, kind="ExternalInput")
    bias_d = nc.dram_tensor("biasT", [128, S * KD], F32, kind="ExternalInput")
    gidx_d = nc.dram_tensor("gidx", [128, NT], I32, kind="ExternalInput")
    vmask_d = nc.dram_tensor("vmask", [128, NT], F32, kind="ExternalInput")
    cmask_d = nc.dram_tensor("cmask", [PP, R], BF16, kind="ExternalInput")
    pmask_d = nc.dram_tensor("pmask", [PP, R], BF16, kind="ExternalInput")
    c0_d = nc.dram_tensor("c0T", [128, NT], F32, kind="ExternalInput")
    out_d = nc.dram_tensor("out", [1, 2], F32, kind="ExternalOutput")
    if DEBUG:
        predT_dbg = nc.dram_tensor(
            "predT_dbg", [128, KD, PP], BF16, kind="ExternalOutput"
        )
        cols_dbg = nc.dram_tensor("cols_dbg", [128, 4 * NT], F32, kind="ExternalOutput")
        res_dbg = nc.dram_tensor("res_dbg", [128, 2 * NT], F32, kind="ExternalOutput")

    with tile.TileContext(nc) as tc:
        with (
            tc.tile_pool(name="const", bufs=1) as const,
            tc.tile_pool(name="spool", bufs=4) as spool,
            tc.tile_pool(name="ps", bufs=4, space="PSUM") as ps_pool,
            tc.tile_pool(name="pt", bufs=2, space="PSUM") as pt_pool,
            tc.tile_pool(name="psf", bufs=1, space="PSUM") as psf_pool,
        ):
            # ---- constants / persistent tiles ----
            encT_sb = const.tile([128, KD, R], BF16)
            nc.sync.dma_start(
                out=encT_sb[:],
                in_=encT_d.ap().rearrange("(k p) e -> p k e", p=128),
            )
            bias_sb = const.tile([128, S * KD], F32)
            nc.sync.dma_start(out=bias_sb[:], in_=bias_d.ap())
            gidx_sb = const.tile([128, NT], I32)
            nc.sync.dma_start(out=gidx_sb[:], in_=gidx_d.ap())
            vmask_sb = const.tile([128, NT], F32)
            nc.sync.dma_start(out=vmask_sb[:], in_=vmask_d.ap())
            c0_sb = const.tile([128, NT], F32)
            nc.sync.dma_start(out=c0_sb[:], in_=c0_d.ap())

            ident = const.tile([128, 128], BF16)
            make_identity(nc, ident[:])
            ones = const.tile([128, 1], F32)
            nc.vector.memset(ones[:], 1.0)

            ctxT_sb = const.tile([128, KD, PP], BF16)
            predT_sb = const.tile([128, KD, PP], BF16)
            # zero the padded prediction columns so stage B stays finite
            nc.vector.memset(predT_sb[:, :, PC:PP], 0.0)

            nmax_sb = const.tile([128, NT], F32)   # negated candidate max
            dots0_sb = const.tile([128, NT], F32)  # positive logit
            sume_sb = const.tile([128, NT], F32)   # sum of M*exp(masked-max)
            lnS_sb = const.tile([128, NT], F32)
            res_sb = const.tile([128, 2 * NT], F32)

            # ---- phase 1: gather ctx rows, cast, transpose ----
            with tc.tile_pool(name="gpool", bufs=3) as gpool:
                for t in range(NT):
                    g = gpool.tile([128, D], BF16)
                    nc.gpsimd.indirect_dma_start(
                        out=g[:],
                        out_offset=None,
                        in_=ctx_d.ap(),
                        in_offset=IndirectOffsetOnAxis(
                            ap=gidx_sb[:, t : t + 1], axis=0
                        ),
                    )
                    for k in range(KD):
                        pt = pt_pool.tile([128, 128], BF16, tag="pt")
                        nc.tensor.transpose(
                            pt[:], g[:, k * 128 : (k + 1) * 128], ident[:]
                        )
                        nc.vector.tensor_copy(
                            ctxT_sb[:, k, t * 128 : (t + 1) * 128], pt[:]
                        )

            # ---- phase 2 (stage A): predT = W^T-contract(ctxT) + bias ----
            with tc.tile_pool(name="wpool", bufs=2) as wpool:
                for s in range(S):
                    w_sb = wpool.tile([128, KD, D], BF16)
                    nc.sync.dma_start(
                        out=w_sb[:],
                        in_=WT_d.ap()[s].rearrange("(k p) j -> p k j", p=128),
                    )
                    lo, ln = PO[s], L[s]
                    for m in range(KD):
                        pa = ps_pool.tile([128, ECH], F32, tag="ps")
                        for k in range(KD):
                            nc.tensor.matmul(
                                pa[:, :ln],
                                lhsT=w_sb[:, k, m * 128 : (m + 1) * 128],
                                rhs=ctxT_sb[:, k, lo : lo + ln],
                                start=(k == 0),
                                stop=(k == KD - 1),
                            )
                        nc.scalar.activation(
                            predT_sb[:, m, lo : lo + ln],
                            pa[:, :ln],
                            mybir.ActivationFunctionType.Identity,
                            bias=bias_sb[:, s * KD + m : s * KD + m + 1],
                            scale=1.0,
                        )

            # ---- phase 3 (stage B + masked softmax stats per p-tile) ----
            with (
                tc.tile_pool(name="mpool", bufs=2) as mpool,
                tc.tile_pool(name="mk", bufs=2) as mkpool,
                tc.tile_pool(name="trash", bufs=1) as trashpool,
            ):
                for mp in range(NT):
                    rows = slice(mp * 128, (mp + 1) * 128)
                    Nt = mpool.tile([128, R], BF16, tag="N")
                    nc.sync.dma_start(out=Nt[:], in_=cmask_d.ap()[rows, :])
                    Pt = mpool.tile([128, R], BF16, tag="P")
                    nc.sync.dma_start(out=Pt[:], in_=pmask_d.ap()[rows, :])

                    masked = mkpool.tile([128, R], F32)
                    for n in range(NE):
                        cols = slice(n * ECH, (n + 1) * ECH)
                        pb = ps_pool.tile([128, ECH], F32, tag="ps")
                        for k in range(KD):
                            nc.tensor.matmul(
                                pb[:],
                                lhsT=predT_sb[:, k, rows],
                                rhs=encT_sb[:, k, cols],
                                start=(k == 0),
                                stop=(k == KD - 1),
                            )
                        # evacuate + apply the -inf candidate mask
                        nc.vector.tensor_add(masked[:, cols], pb[:], Nt[:, cols])

                    # exact positive logit: sum_e P0 * masked
                    scr = trashpool.tile([128, R], F32, tag="scr")
                    nc.gpsimd.tensor_mul(scr[:], masked[:], Pt[:])
                    nc.vector.reduce_sum(
                        dots0_sb[:, mp : mp + 1], scr[:], axis=mybir.AxisListType.X
                    )
                    # negated candidate max
                    nc.vector.reduce_max(
                        nmax_sb[:, mp : mp + 1],
                        masked[:],
                        axis=mybir.AxisListType.X,
                        negate=True,
                    )
                    # sumexp = sum_e exp(masked - max), multiplicity folded
                    # into the mask as ln(M); accumulated by the ACT engine
                    Et = trashpool.tile([128, R], BF16, tag="E")
                    nc.scalar.activation(
                        Et[:],
                        masked[:],
                        mybir.ActivationFunctionType.Exp,
                        bias=nmax_sb[:, mp : mp + 1],
                        scale=1.0,
                        accum_out=sume_sb[:, mp : mp + 1],
                    )

            # ---- phase 4: per-prediction loss/correct, masked, reduced ----
            nc.scalar.activation(
                lnS_sb[:], sume_sb[:], mybir.ActivationFunctionType.Ln
            )
            t1 = spool.tile([128, NT], F32)
            nc.vector.tensor_sub(t1[:], lnS_sb[:], dots0_sb[:])
            t2 = spool.tile([128, NT], F32)
            nc.vector.tensor_sub(t2[:], t1[:], nmax_sb[:])  # + truemax
            lossp = spool.tile([128, NT], F32)
            nc.vector.tensor_add(lossp[:], t2[:], c0_sb[:])  # undo ln(M) on dots0
            tmax = spool.tile([128, NT], F32)
            nc.vector.tensor_scalar_mul(tmax[:], nmax_sb[:], -1.0)
            corrp = spool.tile([128, NT], F32)
            nc.vector.tensor_tensor(
                out=corrp[:], in0=dots0_sb[:], in1=tmax[:], op=mybir.AluOpType.is_ge
            )
            nc.vector.tensor_mul(res_sb[:, 0:NT], lossp[:], vmask_sb[:])
            nc.vector.tensor_mul(res_sb[:, NT : 2 * NT], corrp[:], vmask_sb[:])

            if DEBUG:
                nc.sync.dma_start(out=predT_dbg.ap(), in_=predT_sb[:])
                nc.sync.dma_start(out=cols_dbg.ap()[:, 0:NT], in_=nmax_sb[:])
                nc.sync.dma_start(
                    out=cols_dbg.ap()[:, NT : 2 * NT], in_=dots0_sb[:]
                )
                nc.sync.dma_start(
                    out=cols_dbg.ap()[:, 2 * NT : 3 * NT], in_=sume_sb[:]
                )
                nc.sync.dma_start(
                    out=cols_dbg.ap()[:, 3 * NT : 4 * NT], in_=lnS_sb[:]
                )
                nc.sync.dma_start(out=res_dbg.ap(), in_=res_sb[:])

            # ---- final reduction ----
            fin = const.tile([128, 2], F32)
            nc.vector.reduce_sum(
                fin[:, 0:1], res_sb[:, 0:NT], axis=mybir.AxisListType.X
            )
            nc.vector.reduce_sum(
                fin[:, 1:2], res_sb[:, NT : 2 * NT], axis=mybir.AxisListType.X
            )
            pf = psf_pool.tile([1, 2], F32)
            nc.tensor.matmul(pf[:], lhsT=ones[:], rhs=fin[:], start=True, stop=True)
            out_sb = const.tile([1, 2], F32)
            nc.vector.tensor_copy(out_sb[:], pf[:])
            nc.sync.dma_start(out=out_d.ap(), in_=out_sb[:])

    nc.compile()
    _CACHE["nc"] = nc
    return nc


def _prep_in_maps(contexts, encodings, Wk_w, Wk_b, ctx_idx, cand_idx):
    ctx_flat = np.ascontiguousarray(
        np.asarray(contexts, dtype=np.float32).reshape(R, D)
    ).astype(ml_dtypes.bfloat16)
    encT = np.ascontiguousarray(
        np.asarray(encodings, dtype=np.float32).reshape(R, D).T
    ).astype(ml_dtypes.bfloat16)
    WT = np.ascontiguousarray(
        np.asarray(Wk_w, dtype=np.float32).transpose(0, 2, 1)
    ).astype(ml_dtypes.bfloat16)
    biasT = np.ascontiguousarray(
        np.asarray(Wk_b, dtype=np.float32).reshape(S, KD, 128).transpose(2, 0, 1)
        .reshape(128, S * KD)
    )
    ctx_idx = np.asarray(ctx_idx, dtype=np.int32)
    cand_idx = np.asarray(cand_idx, dtype=np.int32)

    offs = np.concatenate([[0], np.cumsum(STEP_LENS)]).astype(np.int64)

    in_maps = []
    for c in range(N_CORES):
        ci_parts, ki_parts = [], []
        for s in range(S):
            a = int(offs[s]) + c * L[s]
            ci_parts.append(ctx_idx[a : a + L[s]])
            ki_parts.append(cand_idx[a : a + L[s]])
        ci = np.concatenate(ci_parts)                    # [1120]
        ki = np.concatenate(ki_parts, axis=0).astype(np.int64)  # [1120, 17]
        ci_pad = np.zeros(PP, np.int32)
        ci_pad[:PC] = ci
        gidx = np.ascontiguousarray(ci_pad.reshape(NT, 128).T)            # [128, 9]
        vmask = np.ascontiguousarray(
            (np.arange(PP) < PC).astype(np.float32).reshape(NT, 128).T
        )
        prow = np.arange(PC)
        mm = np.zeros((PP, R), np.float32)
        np.add.at(mm, (np.repeat(prow, K17), ki.ravel()), 1.0)
        mm[PC:, 0] = 1.0
        with np.errstate(divide="ignore"):
            cm = np.where(mm > 0, np.log(np.maximum(mm, 1.0)), NEGINF).astype(
                np.float32
            )
        pm = np.zeros((PP, R), np.float32)
        pm[prow, ki[:, 0]] = 1.0
        pm[PC:, 0] = 1.0
        c0 = np.zeros(PP, np.float32)
        # match the bf16 rounding of ln(M) that the device mask carries
        c0[:PC] = (
            np.log(mm[prow, ki[:, 0]])
            .astype(ml_dtypes.bfloat16)
            .astype(np.float32)
        )
        c0T = np.ascontiguousarray(c0.reshape(NT, 128).T)
        in_maps.append(
            {
                "ctx": ctx_flat,
                "encT": encT,
                "WT": WT,
                "biasT": biasT,
                "gidx": gidx,
                "vmask": vmask,
                "cmask": cm.astype(ml_dtypes.bfloat16),
                "pmask": pm.astype(ml_dtypes.bfloat16),
                "c0T": c0T,
            }
        )
    return in_maps


def _install_ntff_hook():
    """Provide antenv.axon_hooks if the image lacks it, so trace=True can
    capture NTFF profiles through the injected libaxon_pjrt.so (mirrors
    trn_boot._ntff_profile_via_ctypes)."""
    import sys
    import types
    import ctypes
    import contextlib
    import os

    try:
        from antenv.axon_hooks import get_axon_ntff_profile_hook  # noqa: F401

        return
    except ImportError:
        pass
    so_path = "/opt/axon/libaxon_pjrt.so"
    if not os.path.exists(so_path):
        return
    lib = ctypes.CDLL(so_path)
    if not hasattr(lib, "axon_start_nrt_profile"):
        return
    lib.axon_start_nrt_profile.argtypes = [
        ctypes.POINTER(ctypes.c_int64),
        ctypes.c_size_t,
    ]
    lib.axon_start_nrt_profile.restype = ctypes.c_int64
    lib.axon_stop_nrt_profile.argtypes = [ctypes.c_char_p]
    lib.axon_stop_nrt_profile.restype = ctypes.c_int64

    @contextlib.contextmanager
    def _hook(output_dir, device_ids):
        import jax

        jax.devices()
        if device_ids:
            ids = (ctypes.c_int64 * len(device_ids))(*device_ids)
            rc = lib.axon_start_nrt_profile(ids, len(device_ids))
        else:
            rc = lib.axon_start_nrt_profile(None, 0)
        if rc != 0:
            raise RuntimeError(f"axon_start_nrt_profile rc={rc}")
        try:
            yield
        finally:
            n = lib.axon_stop_nrt_profile(str(output_dir).encode())
            print(f"ntff profile: {n} file(s) written to {output_dir}")

    mod = types.ModuleType("antenv.axon_hooks")
    mod.get_axon_ntff_profile_hook = lambda: _hook
    mod.set_axon_ntff_profile_hook = lambda h: None
    sys.modules["antenv.axon_hooks"] = mod


def run(inputs, trace=False, **kwargs):
    """Run the SPMD kernel; returns (loss, correct, BassKernelResults)."""
    if trace:
        _install_ntff_hook()
    nc = _build()
    in_maps = _prep_in_maps(**inputs)
    res = run_bass_kernel_spmd(
        nc, in_maps, core_ids=list(range(N_CORES)), trace=trace, **kwargs
    )
    sums = np.stack([r["out"].reshape(2) for r in res.results])  # [8, 2]
    tot = sums.sum(axis=0, dtype=np.float64)
    loss = np.float32(tot[0] / P_TOTAL)
    correct = np.float32(tot[1] / P_TOTAL)
    return loss, correct, res


def kernel(**inputs):
    loss, correct, _ = run(inputs, trace=False)
    return loss, correct
